# revision 13
# baseline (speedup 1.0000x reference)
"""Trainium2 Bass kernel for nn_Encoder segment-reduce.

Reference computation (per sample b):
    cls = onehot(argmax_k outputs[b])            # [K, HW]
    sizes = cls.sum(HW) + 0.01                   # [K]
    feat_set = feats[b] @ cls.T / sizes          # [F, K]
    out[b] = w_proj @ feat_set + bias            # [E, K]

Kernel strategy (pure data parallel: 1 sample per NeuronCore, 8 cores):
    Since the division by sizes and the projection are both linear, reorder:
        out[b].T[k, e] = (onehot.T @ (feats.T @ wT))[k, e] / sizes[k] + bias[e]
    The inner matmul projT[hw, e] = feats_chunk.T @ wT uses feats tiles as the
    matmul's STATIONARY operand in their natural [F, HW] layout, so no
    transpose of the 32MB feats tensor is ever needed.  The segment-reduce
    then contracts projT (hw on partitions) against the onehot matrix
    (hw on partitions) accumulating [K, E] in PSUM across all hw chunks.

    argmax one-hot: PE-transpose outputs chunks [K,128] -> [128,K], then
    rowmax (DVE reduce) + is_equal compare.  sizes come from onehot.T @ ones
    accumulated in PSUM (exact fp32 counts).

All matmuls run in float32r (full-rate fp32 mode on TRN2 PE for N>=256).
"""

import numpy as np

import concourse.bacc as bacc
import concourse.bass as bass
import concourse.mybir as mybir
import concourse.tile as tile
from concourse.bass import ds, ts
from concourse.bass_utils import run_bass_kernel_spmd
from concourse.masks import make_identity

# Problem shapes (hardcoded per contract)
B = 8
K = 21
H = 64
W = 64
HW = H * W            # 4096
F = 2048
E = 256
P = 128
FC = F // P           # 16 f-chunks
HW_BLK = 512          # hw columns per DMA block
N_BLK = HW // HW_BLK  # 8
SUB = HW_BLK // P     # 4 chunks of 128 per block
N_T = HW // P         # 32 hw chunks
N_CORES = 8

F32 = mybir.dt.float32
F32R = mybir.dt.float32r


def build_module():
    nc = bacc.Bacc("TRN2", target_bir_lowering=False, debug=False)

    outputs_d = nc.dram_tensor("outputs_in", [K, HW], F32, kind="ExternalInput")
    feats_d = nc.dram_tensor("feats_in", [F, HW], F32R, kind="ExternalInput")
    wT_d = nc.dram_tensor("wT_in", [F, E], F32R, kind="ExternalInput")
    bias_d = nc.dram_tensor("bias_in", [E], F32, kind="ExternalInput")
    out_d = nc.dram_tensor("out", [K, E], F32, kind="ExternalOutput")

    with tile.TileContext(nc) as tc:
        with (
            tc.tile_pool(name="consts", bufs=1) as consts,
            tc.tile_pool(name="feats", bufs=3) as feats_pool,
            tc.tile_pool(name="small", bufs=4) as small,
            tc.tile_pool(name="projT", bufs=3) as projT_pool,
            tc.tile_pool(name="outp", bufs=1) as outp,
            tc.tile_pool(name="ps_tr", bufs=2, space="PSUM") as ps_tr,
            tc.tile_pool(name="ps_proj", bufs=4, space="PSUM") as ps_proj,
            tc.tile_pool(name="ps_out", bufs=1, space="PSUM") as ps_out_pool,
        ):
            # Small input DMAs ride the scalar HWDGE queue so they are not
            # stuck behind the 4MB feats transfers on the sync queue.
            outputs_sb = consts.tile([K, HW], F32)
            nc.scalar.dma_start(out=outputs_sb, in_=outputs_d.ap())
            ident = consts.tile([P, P], F32)
            make_identity(nc, ident)
            ones_f = consts.tile([P, 2], F32)
            nc.vector.memset(ones_f, 1.0)

            wT_sb = consts.tile([P, FC, E], F32R)
            nc.scalar.dma_start(
                out=wT_sb, in_=wT_d.ap().rearrange("(fc p) e -> p fc e", p=P)
            )
            bias_ap = bias_d.ap()
            bias_bc = consts.tile([K, E], F32)
            nc.scalar.dma_start(
                out=bias_bc,
                in_=bass.AP(
                    tensor=bias_ap.tensor, offset=bias_ap.offset, ap=[[0, K], [1, E]]
                ),
            )

            # psum_out columns [0:E) accumulate onehot.T @ projT; columns
            # [E:E+2) accumulate onehot.T @ 1 = the class sizes.
            psum_out = ps_out_pool.tile([K, E + 2], F32)
            oh_all = consts.tile([P, N_T, K], F32R)

            # Phase 1: onehot construction
            for t in range(N_T):
                tr = ps_tr.tile([P, K], F32)
                nc.tensor.transpose(tr, outputs_sb[:, ts(t, P)], ident[:K, :K])
                rowmax = small.tile([P, 1], F32)
                nc.vector.tensor_reduce(
                    rowmax, tr, mybir.AxisListType.X, mybir.AluOpType.max
                )
                nc.vector.tensor_scalar(
                    out=oh_all[:, t, :],
                    in0=tr,
                    scalar1=rowmax,
                    scalar2=None,
                    op0=mybir.AluOpType.is_equal,
                )

            # Phase 2: projection (feats stationary) + segment accumulate
            feats_r = feats_d.ap().rearrange("(fc p) hw -> p fc hw", p=P)
            for g in range(N_BLK):
                fg = feats_pool.tile([P, FC, HW_BLK], F32R)
                nc.sync.dma_start(out=fg, in_=feats_r[:, :, ds(g * HW_BLK, HW_BLK)])
                for s in range(SUB):
                    t = g * SUB + s
                    pt = ps_proj.tile([P, E], F32)
                    for fc in range(FC):
                        nc.tensor.matmul(
                            pt,
                            lhsT=fg[:, fc, ts(s, P)],
                            rhs=wT_sb[:, fc, :],
                            start=(fc == 0),
                            stop=(fc == FC - 1),
                        )
                    pts = projT_pool.tile([P, E + 2], F32R)
                    nc.vector.tensor_copy(pts[:, 0:E], pt)
                    nc.vector.tensor_copy(pts[:, E : E + 2], ones_f)
                    nc.tensor.matmul(
                        psum_out,
                        lhsT=oh_all[:, t, :],
                        rhs=pts,
                        start=(t == 0),
                        stop=(t == N_T - 1),
                    )

            # Phase 3: scale by 1/sizes, add bias, store
            sizes_sb = small.tile([K, 1], F32, tag="sizes")
            nc.vector.tensor_scalar_add(sizes_sb, psum_out[:, E : E + 1], 0.01)
            recip = small.tile([K, 1], F32, tag="recip")
            nc.vector.reciprocal(recip, sizes_sb)
            out_sb = outp.tile([K, E], F32)
            nc.vector.scalar_tensor_tensor(
                out=out_sb,
                in0=psum_out[:, 0:E],
                scalar=recip,
                in1=bias_bc,
                op0=mybir.AluOpType.mult,
                op1=mybir.AluOpType.add,
            )
            nc.scalar.dma_start(out=out_d.ap(), in_=out_sb)

    nc.compile()
    return nc


_CACHE = {}


def make_in_maps(outputs, feats, w_proj, b_proj):
    outputs = np.ascontiguousarray(np.asarray(outputs, dtype=np.float32))
    feats = np.ascontiguousarray(np.asarray(feats, dtype=np.float32))
    wT = np.ascontiguousarray(np.asarray(w_proj, dtype=np.float32).T)
    bias = np.ascontiguousarray(np.asarray(b_proj, dtype=np.float32))
    return [
        {
            "outputs_in": outputs[b].reshape(K, HW),
            "feats_in": feats[b].reshape(F, HW),
            "wT_in": wT,
            "bias_in": bias,
        }
        for b in range(B)
    ]


def kernel(outputs, feats, w_proj, b_proj, _trace=False, _trace_kwargs=None):
    if "nc" not in _CACHE:
        _CACHE["nc"] = build_module()
    nc = _CACHE["nc"]
    in_maps = make_in_maps(outputs, feats, w_proj, b_proj)
    res = run_bass_kernel_spmd(
        nc,
        in_maps,
        core_ids=list(range(N_CORES)),
        trace=_trace,
        **(_trace_kwargs or {}),
    )
    out = np.stack([np.asarray(r["out"]).T for r in res.results])
    if _trace:
        _CACHE["last_results"] = res
    return out


# revision 14
# speedup vs baseline: 1.2770x; 1.2770x over previous
"""Trainium2 Bass kernel for nn_Encoder segment-reduce.

Reference computation (per sample b):
    cls = onehot(argmax_k outputs[b])            # [K, HW]
    sizes = cls.sum(HW) + 0.01                   # [K]
    feat_set = feats[b] @ cls.T / sizes          # [F, K]
    out[b] = w_proj @ feat_set + bias            # [E, K]

Kernel strategy (pure data parallel: 1 sample per NeuronCore, 8 cores):
    Since the division by sizes and the projection are both linear, reorder:
        out[b].T[k, e] = (onehot.T @ (feats.T @ wT))[k, e] / sizes[k] + bias[e]
    The inner matmul projT[hw, e] = feats_chunk.T @ wT uses feats tiles as the
    matmul's STATIONARY operand in their natural [F, HW] layout, so no
    transpose of the 32MB feats tensor is ever needed.  The segment-reduce
    then contracts projT (hw on partitions) against the onehot matrix
    (hw on partitions), accumulating [K, E+2] in PSUM across all hw chunks —
    the two extra `ones` columns appended to projT make the same matmul
    accumulate the class sizes for free.

    argmax one-hot: PE-transpose outputs chunks [K,128] -> [128,K], then
    rowmax (DVE reduce) + is_equal compare.

dtype: "f32r" (full fp32 DMA, float32r full-rate matmuls, rel err ~2e-4) or
"bf16" (host-cast feats/wT to bf16: half the HBM traffic, rel err ~5e-3).
"""

import numpy as np

import concourse.bacc as bacc
import concourse.bass as bass
import concourse.mybir as mybir
import concourse.tile as tile
from concourse.bass import ds, ts
from concourse.bass_utils import run_bass_kernel_spmd
from concourse.masks import make_identity

# Problem shapes (hardcoded per contract)
B = 8
K = 21
H = 64
W = 64
HW = H * W            # 4096
F = 2048
E = 256
P = 128
FC = F // P           # 16 f-chunks
N_CORES = 8

F32 = mybir.dt.float32
F32R = mybir.dt.float32r
BF16 = mybir.dt.bfloat16

DTYPE = "bf16"        # "bf16" or "f32r"


def build_module(dtype=DTYPE, hw_blk=256, feats_bufs=4):
    n_blk = HW // hw_blk
    sub = hw_blk // P
    n_t = HW // P

    mm_dt = BF16 if dtype == "bf16" else F32R
    nc = bacc.Bacc("TRN2", target_bir_lowering=False, debug=False)

    outputs_d = nc.dram_tensor("outputs_in", [K, HW], F32, kind="ExternalInput")
    feats_d = nc.dram_tensor("feats_in", [F, HW], mm_dt, kind="ExternalInput")
    wT_d = nc.dram_tensor("wT_in", [F, E], mm_dt, kind="ExternalInput")
    bias_d = nc.dram_tensor("bias_in", [E], F32, kind="ExternalInput")
    out_d = nc.dram_tensor("out", [K, E], F32, kind="ExternalOutput")

    with tile.TileContext(nc) as tc:
        with (
            tc.tile_pool(name="consts", bufs=1) as consts,
            tc.tile_pool(name="feats", bufs=feats_bufs) as feats_pool,
            tc.tile_pool(name="small", bufs=4) as small,
            tc.tile_pool(name="projT", bufs=3) as projT_pool,
            tc.tile_pool(name="outp", bufs=1) as outp,
            tc.tile_pool(name="ps_tr", bufs=2, space="PSUM") as ps_tr,
            tc.tile_pool(name="ps_proj", bufs=4, space="PSUM") as ps_proj,
            tc.tile_pool(name="ps_out", bufs=1, space="PSUM") as ps_out_pool,
        ):
            # outputs first on the sync HWDGE queue (phase 1 needs it ASAP);
            # feats blocks follow on the same queue.  wT/bias ride the gpsimd
            # SWDGE queue so they land in parallel with the feats stream.
            outputs_sb = consts.tile([K, HW], F32)
            nc.sync.dma_start(out=outputs_sb, in_=outputs_d.ap())

            feats_r = feats_d.ap().rearrange("(fc p) hw -> p fc hw", p=P)
            fgs = []
            for g in range(n_blk):
                fg = feats_pool.tile([P, FC, hw_blk], mm_dt)
                nc.sync.dma_start(out=fg, in_=feats_r[:, :, ds(g * hw_blk, hw_blk)])
                fgs.append(fg)

            ident = consts.tile([P, P], F32)
            make_identity(nc, ident)
            ones_f = consts.tile([P, 2], F32)
            nc.vector.memset(ones_f, 1.0)

            wT_sb = consts.tile([P, FC, E], mm_dt)
            nc.gpsimd.dma_start(
                out=wT_sb, in_=wT_d.ap().rearrange("(fc p) e -> p fc e", p=P)
            )
            bias_ap = bias_d.ap()
            bias_bc = consts.tile([K, E], F32)
            nc.gpsimd.dma_start(
                out=bias_bc,
                in_=bass.AP(
                    tensor=bias_ap.tensor, offset=bias_ap.offset, ap=[[0, K], [1, E]]
                ),
            )

            # psum_out columns [0:E) accumulate onehot.T @ projT; columns
            # [E:E+2) accumulate onehot.T @ 1 = the class sizes.
            psum_out = ps_out_pool.tile([K, E + 2], F32)
            oh_all = consts.tile([P, n_t, K], mm_dt)

            # Phase 1: onehot construction
            for t in range(n_t):
                tr = ps_tr.tile([P, K], F32)
                nc.tensor.transpose(tr, outputs_sb[:, ts(t, P)], ident[:K, :K])
                rowmax = small.tile([P, 1], F32)
                nc.vector.tensor_reduce(
                    rowmax, tr, mybir.AxisListType.X, mybir.AluOpType.max
                )
                nc.vector.tensor_scalar(
                    out=oh_all[:, t, :],
                    in0=tr,
                    scalar1=rowmax,
                    scalar2=None,
                    op0=mybir.AluOpType.is_equal,
                )

            # Phase 2: projection (feats stationary) + segment accumulate
            for g in range(n_blk):
                fg = fgs[g]
                for s in range(sub):
                    t = g * sub + s
                    pt = ps_proj.tile([P, E], F32)
                    for fc in range(FC):
                        nc.tensor.matmul(
                            pt,
                            lhsT=fg[:, fc, ts(s, P)],
                            rhs=wT_sb[:, fc, :],
                            start=(fc == 0),
                            stop=(fc == FC - 1),
                        )
                    pts = projT_pool.tile([P, E + 2], mm_dt)
                    nc.vector.tensor_copy(pts[:, 0:E], pt)
                    nc.vector.tensor_copy(pts[:, E : E + 2], ones_f)
                    nc.tensor.matmul(
                        psum_out,
                        lhsT=oh_all[:, t, :],
                        rhs=pts,
                        start=(t == 0),
                        stop=(t == n_t - 1),
                    )

            # Phase 3: scale by 1/sizes, add bias, store
            sizes_sb = small.tile([K, 1], F32, tag="sizes")
            nc.vector.tensor_scalar_add(sizes_sb, psum_out[:, E : E + 1], 0.01)
            recip = small.tile([K, 1], F32, tag="recip")
            nc.vector.reciprocal(recip, sizes_sb)
            out_sb = outp.tile([K, E], F32)
            nc.vector.scalar_tensor_tensor(
                out=out_sb,
                in0=psum_out[:, 0:E],
                scalar=recip,
                in1=bias_bc,
                op0=mybir.AluOpType.mult,
                op1=mybir.AluOpType.add,
            )
            nc.sync.dma_start(out=out_d.ap(), in_=out_sb)

    nc.compile()
    return nc


_CACHE = {}


def make_in_maps(outputs, feats, w_proj, b_proj, dtype=DTYPE):
    import ml_dtypes

    mm_np = ml_dtypes.bfloat16 if dtype == "bf16" else np.float32
    outputs = np.ascontiguousarray(np.asarray(outputs, dtype=np.float32))
    feats = np.ascontiguousarray(np.asarray(feats, dtype=np.float32).astype(mm_np))
    wT = np.ascontiguousarray(np.asarray(w_proj, dtype=np.float32).T.astype(mm_np))
    bias = np.ascontiguousarray(np.asarray(b_proj, dtype=np.float32))
    return [
        {
            "outputs_in": outputs[b].reshape(K, HW),
            "feats_in": feats[b].reshape(F, HW),
            "wT_in": wT,
            "bias_in": bias,
        }
        for b in range(B)
    ]


def kernel(outputs, feats, w_proj, b_proj, _trace=False, _trace_kwargs=None,
           _dtype=DTYPE, _build_kwargs=None):
    key = (_dtype, tuple(sorted((_build_kwargs or {}).items())))
    if key not in _CACHE:
        _CACHE[key] = build_module(dtype=_dtype, **(_build_kwargs or {}))
    nc = _CACHE[key]
    in_maps = make_in_maps(outputs, feats, w_proj, b_proj, dtype=_dtype)
    res = run_bass_kernel_spmd(
        nc,
        in_maps,
        core_ids=list(range(N_CORES)),
        trace=_trace,
        **(_trace_kwargs or {}),
    )
    out = np.stack([np.asarray(r["out"]).T for r in res.results])
    if _trace:
        _CACHE["last_results"] = res
    return out
